# revision 29
# baseline (speedup 1.0000x reference)
"""EGT (edge-gated transformer) layer as a Bass/Tile kernel on 8 trn2 cores.

Problem (hardcoded shapes): B=4, N=1024, H=8, D=32.
  QKV [4,1024,768], E [4,1024,1024,8], G [4,1024,1024,8]
  returns (V_att [4,1024,256], H_hat [4,1024,1024,8], A_tild [4,1024,1024,8])

Sharding: core c handles batch b=c//2 and query-row half l0=(c%2)*512.
Each core reads QKV[b] (3MB) + E,G row-slices (16MB each) and writes
H_hat, A_tild row-slices (16MB each) + V_att slice (0.5MB).

Per-core dataflow (l-tiles of 128 rows x m-halves of 512 keys):
  - setup: load V natively; build K^T,Q^T ([d, m|l] per head) via PE transposes
  - mm1: A = Q.K^T per head (32-partition contraction)
  - clip via one 2-op tensor_scalar (min,max) PSUM->SBUF, h-interleaved layout
  - H_hat = clip(A) + E with E added by a gpsimd accumulate-DMA (CCE-chunked)
  - exp/tanh on ACT (strided in, flat slab out) with fused accum_out row sums;
    sigmoid(g) == (1+tanh(g/2))/2 keeps everything in one ACT table set
  - W = exp*(1+tanh) kept in flat per-head slabs; 1/denom commutes through
    the value matmul, so mm2 consumes W directly (contiguous PE transposes,
    no dependency on the cross-half denominator)
  - A_tild = W * (0.5/denom) interleaved for DRAM, off the mm2 critical path
  - V_att = psum(W^T.V) * (0.5/denom) * log1p(degrees) at the end (single Ln)
"""

import sys

if "/opt/trn_rl_repo" not in sys.path:
    sys.path.insert(0, "/opt/trn_rl_repo")

import contextlib
from contextlib import ExitStack

import numpy as np

import concourse.bass as bass
import concourse.tile as tile
from concourse import bacc, masks, mybir
from concourse._compat import with_exitstack
from concourse.bass_utils import run_bass_kernel_spmd

F32 = mybir.dt.float32
AF = mybir.ActivationFunctionType
ALU = mybir.AluOpType

B, N, H, D = 4, 1024, 8, 32
P = 128            # sbuf partitions / l-tile rows
LCORE = 512        # l rows per core
NLT = LCORE // P   # l-tiles per core (4)
MH = 512           # m-half size
NMH = N // MH      # m halves (2)
NCH = MH // P      # 128-chunks per m half (4)
NT = N // P        # m tiles over full N (8)
CLIP = 5.0


@with_exitstack
def egt_body(ctx: ExitStack, tc: tile.TileContext, outs, ins, repeat=1,
             alias_rows=False, variant=()):
    nc = tc.nc
    h_out, a_out, v_out = outs
    qkv, q_in, e_in, g_in = ins

    def rows(i):
        # timing-alias mode: all l-tiles hit the same 128 DRAM rows so the
        # timing build can use tiny I/O tensors (identical instruction stream)
        return slice(0, P) if alias_rows else slice(i * P, (i + 1) * P)

    rep_ctx = tc.For_i(0, repeat, 1) if repeat > 1 else None

    const = ctx.enter_context(tc.tile_pool(name="const", bufs=1))
    setup = ctx.enter_context(tc.tile_pool(name="setup", bufs=2))
    hpool = ctx.enter_context(tc.tile_pool(name="hpool", bufs=2))
    gpool = ctx.enter_context(tc.tile_pool(name="gpool", bufs=2))
    apool = ctx.enter_context(tc.tile_pool(name="apool", bufs=2))
    wpool = ctx.enter_context(tc.tile_pool(name="wpool", bufs=3))
    upool = ctx.enter_context(tc.tile_pool(name="upool", bufs=3))
    atpool = ctx.enter_context(tc.tile_pool(name="atpool", bufs=2))
    smalls = ctx.enter_context(tc.tile_pool(name="smalls", bufs=1))
    ps_mm1 = ctx.enter_context(tc.tile_pool(name="ps_mm1", bufs=2, space="PSUM"))
    ps_tr = ctx.enter_context(tc.tile_pool(name="ps_tr", bufs=2, space="PSUM"))
    ps_v = ctx.enter_context(tc.tile_pool(name="ps_v", bufs=2, space="PSUM"))

    ident = const.tile([P, P], F32)
    masks.make_identity(nc, ident[:])
    ln_bias = const.tile([P, 1], F32)
    nc.gpsimd.memset(ln_bias[:], 1.0 + N / 2.0)

    with rep_ctx if rep_ctx is not None else contextlib.nullcontext():
        # ---- setup: V native, K^T / Q^T per head ----
        v_sb = const.tile([P, NT * D * H], F32, tag="v_sb")
        # packed transposed K/Q: partition 32*g + d holds head h = 2*g + hh
        k_t = const.tile([P, 2 * N], F32, tag="k_t")      # [32g+d, hh*N + m]
        q_t = const.tile([P, 2 * LCORE], F32, tag="q_t")  # [32g+d, hh*LCORE + l]

        for t in range(NT):
            kv_nat = setup.tile([P, 2 * D * H], F32, tag="kv_nat")
            nc.sync.dma_start(kv_nat[:], qkv[t * P : (t + 1) * P, D * H : 3 * D * H])
            nc.vector.tensor_copy(
                v_sb[:, t * D * H : (t + 1) * D * H], kv_nat[:, D * H : 2 * D * H]
            )
            # one [128,128] transpose per hh gathers heads {hh, hh+2, hh+4,
            # hh+6}: out partition 32*g + d <- column d*H + (2g + hh)
            kv_v = kv_nat[:, 0 : D * H].rearrange("p (d h) -> p h d", h=H)
            pst = ps_tr.tile([P, 2 * P], F32, tag="pst")
            stage = setup.tile([P, 2 * P], F32, tag="stage")
            for hh in range(2):
                nc.vector.tensor_copy(
                    stage[:, hh * P : (hh + 1) * P], kv_v[:, hh::2, :]
                )
                nc.tensor.transpose(
                    pst[:, hh * P : (hh + 1) * P],
                    stage[:, hh * P : (hh + 1) * P],
                    ident[:],
                )
            nc.scalar.copy(
                k_t[:].rearrange("p (hh m) -> p hh m", hh=2)[:, :, t * P : (t + 1) * P],
                pst[:].rearrange("p (hh c) -> p hh c", hh=2),
            )

        for i in range(NLT):
            q_nat = setup.tile([P, D * H], F32, tag="q_nat")
            nc.sync.dma_start(q_nat[:], q_in[rows(i), :])
            q_v = q_nat[:].rearrange("p (d h) -> p h d", h=H)
            pst = ps_tr.tile([P, 2 * P], F32, tag="pst")
            stage = setup.tile([P, 2 * P], F32, tag="stage")
            for hh in range(2):
                nc.vector.tensor_copy(
                    stage[:, hh * P : (hh + 1) * P], q_v[:, hh::2, :]
                )
                nc.tensor.transpose(
                    pst[:, hh * P : (hh + 1) * P],
                    stage[:, hh * P : (hh + 1) * P],
                    ident[:],
                )
            nc.scalar.copy(
                q_t[:].rearrange("p (hh l) -> p hh l", hh=2)[
                    :, :, i * P : (i + 1) * P
                ],
                pst[:].rearrange("p (hh c) -> p hh c", hh=2),
            )

        # per-core running stats
        sth_all = smalls.tile([P, NLT * H], F32, tag="sth")     # sum tanh
        hr_all = smalls.tile([P, NLT * H], F32, tag="hr_all")   # 0.5/denom
        vraw = smalls.tile([P, NLT * D * H], F32, tag="vraw")   # unscaled V_att

        for i in range(NLT):
            den = smalls.tile([P, NMH * H], F32, tag="den")
            sth = smalls.tile([P, NMH * H], F32, tag="sthp")
            psv = ps_v.tile([P, H * D], F32, tag="psv")
            w_tiles = []
            for s in range(NMH):
                hsb = hpool.tile([P, MH * H], F32, tag="hsb")
                gsb = gpool.tile([P, MH * H], F32, tag="gsb")
                wf = wpool.tile([P, MH * H], F32, tag="wf")   # exp -> W slabs
                uth0 = upool.tile([P, 4 * MH], F32, tag="uth")
                uth1 = upool.tile([P, 4 * MH], F32, tag="uth")
                uths = [uth0, uth1]
                w_tiles.append(wf)
                nc.sync.dma_start(
                    gsb[:], g_in[rows(i), s * MH * H : (s + 1) * MH * H]
                )
                # mm1 (head pairs share a 2-bank psum tile) + wide 2-op clip
                # into the interleaved layout via a 3D output view
                hsb_r = hsb[:].rearrange("p (m h) -> p h m", h=H)
                for g in range(4):
                    mm = ps_mm1.tile([P, 2 * MH], F32, tag="mm1")
                    for hh in range(2):
                        nc.tensor.matmul(
                            mm[:, hh * MH : (hh + 1) * MH],
                            q_t[
                                D * g : D * (g + 1),
                                hh * LCORE + i * P : hh * LCORE + (i + 1) * P,
                            ],
                            k_t[
                                D * g : D * (g + 1),
                                hh * N + s * MH : hh * N + (s + 1) * MH,
                            ],
                            start=True,
                            stop=True,
                            tile_position=(D * g, 0),
                        )
                    nc.vector.tensor_scalar(
                        out=hsb_r[:, 2 * g : 2 * g + 2, :],
                        in0=mm[:],
                        scalar1=CLIP,
                        scalar2=-CLIP,
                        op0=ALU.min,
                        op1=ALU.max,
                    )
                # H_hat = clip + E via accumulate DMA (CCE adder caps at 2048
                # elements per descriptor row -> chunk)
                if "no_accum" not in variant:
                    for o in range(0, MH * H, 2048):
                        nc.gpsimd.dma_start(
                            hsb[:, o : o + 2048],
                            e_in[rows(i), s * MH * H + o : s * MH * H + o + 2048],
                            accum_op=ALU.add,
                        )
                if "no_hout" not in variant:
                    nc.gpsimd.dma_start(
                        h_out[rows(i), s * MH * H : (s + 1) * MH * H], hsb[:]
                    )
                # per-head exp/tanh: strided-in -> flat slab out, fused
                # accum_out row sums (denominator and degrees for free)
                for h in range(H):
                    nc.scalar.activation(
                        wf[:, h * MH : (h + 1) * MH],
                        hsb[:, h::H],
                        AF.Exp,
                        accum_out=den[:, s * H + h : s * H + h + 1],
                    )
                    nc.scalar.activation(
                        uths[h // 4][:, (h % 4) * MH : (h % 4 + 1) * MH],
                        gsb[:, h::H],
                        AF.Tanh,
                        scale=0.5,
                        accum_out=sth[:, s * H + h : s * H + h + 1],
                    )
                # W = exp*(1+tanh) = exp + exp*tanh, built on gpsimd in
                # half-tile chunks (W lands in-place in wf; uth is scratch)
                if "no_gpsw" not in variant:
                    for q in range(2):
                        cs = slice(q * 4 * MH, (q + 1) * 4 * MH)
                        u = uths[q]
                        nc.gpsimd.tensor_tensor(u[:], u[:], wf[:, cs], ALU.mult)
                        nc.gpsimd.tensor_tensor(wf[:, cs], wf[:, cs], u[:], ALU.add)
                # transposes of flat W chunks + mm2 (no denom dependency)
                for h in (() if "no_pe2" in variant else range(H)):
                    pst = ps_tr.tile([P, NCH * P], F32, tag="pst")
                    for c in range(NCH):
                        nc.tensor.transpose(
                            pst[:, c * P : (c + 1) * P],
                            wf[:, h * MH + c * P : h * MH + (c + 1) * P],
                            ident[:],
                        )
                    at_sb = atpool.tile([P, NCH * P], F32, tag="at_sb")
                    nc.vector.tensor_copy(at_sb[:], pst[:])
                    for c in range(NCH):
                        t = s * NCH + c  # global m tile index
                        # one accumulation group spans the whole psv bank
                        nc.tensor.matmul(
                            psv[:, h * D : (h + 1) * D],
                            at_sb[:, c * P : (c + 1) * P],
                            v_sb[:, t * D * H + h : (t + 1) * D * H : H],
                            start=(s == 0 and h == 0 and c == 0),
                            stop=(s == NMH - 1 and h == H - 1 and c == NCH - 1),
                        )

            # 0.5/denom over both halves
            denom = smalls.tile([P, H], F32, tag="denom")
            nc.vector.tensor_add(denom[:], den[:, 0:H], den[:, H : 2 * H])
            nc.vector.tensor_add(
                sth_all[:, i * H : (i + 1) * H], sth[:, 0:H], sth[:, H : 2 * H]
            )
            half_r = hr_all[:, i * H : (i + 1) * H]
            nc.vector.reciprocal(half_r, denom[:])
            nc.vector.tensor_scalar_mul(half_r, half_r, 0.5)

            # A_tild = W * (0.5/denom), interleaved for DRAM ([128,2048] chunks)
            QW = MH * H // 2
            for s in (() if "no_aout" in variant else range(NMH)):
                wf = w_tiles[s]
                for q in range(2):
                    asb = apool.tile([P, QW], F32, tag="asb")
                    mq = MH // 2  # m-columns per chunk
                    for h in range(H):
                        nc.vector.tensor_scalar(
                            out=asb[:, h::H],
                            in0=wf[:, h * MH + q * mq : h * MH + (q + 1) * mq],
                            scalar1=half_r[:, h : h + 1],
                            scalar2=None,
                            op0=ALU.mult,
                        )
                    nc.gpsimd.dma_start(
                        a_out[rows(i), s * MH * H + q * QW : s * MH * H + (q + 1) * QW],
                        asb[:],
                    )
            # evict raw V_att (scaled at the end)
            if "no_pe2" in variant:
                nc.vector.tensor_copy(
                    vraw[:, i * D * H : (i + 1) * D * H], w_tiles[0][:, 0 : D * H]
                )
            else:
                nc.scalar.copy(vraw[:, i * D * H : (i + 1) * D * H], psv[:])

        # ---- final: V_att = vraw * (0.5/denom) * log1p(N/2 + 0.5*sum tanh) ----
        lnsc = smalls.tile([P, NLT * H], F32, tag="lnsc")
        nc.scalar.activation(lnsc[:], sth_all[:], AF.Ln, scale=0.5, bias=ln_bias[:])
        nc.vector.tensor_mul(lnsc[:], lnsc[:], hr_all[:])
        vout_sb = smalls.tile([P, NLT * D * H], F32, tag="vout_sb")
        for i in range(NLT):
            for h in range(H):
                nc.vector.tensor_scalar(
                    out=vout_sb[:, i * D * H + h : (i + 1) * D * H : H],
                    in0=vraw[:, i * D * H + h * D : i * D * H + (h + 1) * D],
                    scalar1=lnsc[:, i * H + h : i * H + h + 1],
                    scalar2=None,
                    op0=ALU.mult,
                )
            nc.gpsimd.dma_start(
                v_out[rows(i), :], vout_sb[:, i * D * H : (i + 1) * D * H]
            )


_programs = {}


def build_program(repeat=1, alias_rows=False, variant=()):
    key = (repeat, alias_rows, tuple(sorted(variant)))
    if key in _programs:
        return _programs[key]
    RR = P if alias_rows else LCORE
    nc = bacc.Bacc("TRN2", target_bir_lowering=False, debug=False, num_devices=8)
    qkv = nc.dram_tensor("qkv", [N, 3 * D * H], F32, kind="ExternalInput").ap()
    q_in = nc.dram_tensor("q_in", [RR, D * H], F32, kind="ExternalInput").ap()
    e_in = nc.dram_tensor("e_in", [RR, N * H], F32, kind="ExternalInput").ap()
    g_in = nc.dram_tensor("g_in", [RR, N * H], F32, kind="ExternalInput").ap()
    h_out = nc.dram_tensor("h_out", [RR, N * H], F32, kind="ExternalOutput").ap()
    a_out = nc.dram_tensor("a_out", [RR, N * H], F32, kind="ExternalOutput").ap()
    v_out = nc.dram_tensor("v_out", [RR, D * H], F32, kind="ExternalOutput").ap()
    with tile.TileContext(nc) as tc:
        egt_body(tc, (h_out, a_out, v_out), (qkv, q_in, e_in, g_in), repeat=repeat,
                 alias_rows=alias_rows, variant=variant)
    nc.compile()
    _programs[key] = nc
    return nc


def kernel(QKV, E, G, repeat=1, _timing_out=None):
    QKV = np.ascontiguousarray(np.asarray(QKV, dtype=np.float32))
    E = np.ascontiguousarray(np.asarray(E, dtype=np.float32))
    G = np.ascontiguousarray(np.asarray(G, dtype=np.float32))
    assert QKV.shape == (B, N, 3 * D * H)
    assert E.shape == (B, N, N, H) and G.shape == (B, N, N, H)

    nc = build_program(repeat)
    in_maps = []
    for c in range(8):
        b, lh = c // 2, c % 2
        sl = slice(lh * LCORE, (lh + 1) * LCORE)
        in_maps.append(
            {
                "qkv": QKV[b],
                "q_in": np.ascontiguousarray(QKV[b, sl, 0 : D * H]),
                "e_in": np.ascontiguousarray(E[b, sl].reshape(LCORE, N * H)),
                "g_in": np.ascontiguousarray(G[b, sl].reshape(LCORE, N * H)),
            }
        )
    import time

    t0 = time.perf_counter()
    res = run_bass_kernel_spmd(nc, in_maps, list(range(8))).results
    t1 = time.perf_counter()
    if _timing_out is not None:
        _timing_out.append(t1 - t0)

    V_att = np.empty((B, N, D * H), np.float32)
    H_hat = np.empty((B, N, N, H), np.float32)
    A_tild = np.empty((B, N, N, H), np.float32)
    for c, r in enumerate(res):
        b, lh = c // 2, c % 2
        sl = slice(lh * LCORE, (lh + 1) * LCORE)
        V_att[b, sl] = r["v_out"]
        H_hat[b, sl] = r["h_out"].reshape(LCORE, N, H)
        A_tild[b, sl] = r["a_out"].reshape(LCORE, N, H)
    return V_att, H_hat, A_tild


# revision 30
# speedup vs baseline: 1.1963x; 1.1963x over previous
"""EGT (edge-gated transformer) layer as a Bass/Tile kernel on 8 trn2 cores.

Problem (hardcoded shapes): B=4, N=1024, H=8, D=32.
  QKV [4,1024,768], E [4,1024,1024,8], G [4,1024,1024,8]
  returns (V_att [4,1024,256], H_hat [4,1024,1024,8], A_tild [4,1024,1024,8])

Sharding: core c handles batch b=c//2 and query-row half l0=(c%2)*512.
Each core reads QKV[b] (3MB) + E,G row-slices (16MB each) and writes
H_hat, A_tild row-slices (16MB each) + V_att slice (0.5MB).

Per-core dataflow (l-tiles of 128 rows x m-halves of 512 keys):
  - setup: load V natively; build K^T,Q^T ([d, m|l] per head) via PE transposes
  - mm1: A = Q.K^T per head (32-partition contraction)
  - clip via one 2-op tensor_scalar (min,max) PSUM->SBUF, h-interleaved layout
  - H_hat = clip(A) + E with E added by a gpsimd accumulate-DMA (CCE-chunked)
  - exp/tanh on ACT (strided in, flat slab out) with fused accum_out row sums;
    sigmoid(g) == (1+tanh(g/2))/2 keeps everything in one ACT table set
  - W = exp*(1+tanh) kept in flat per-head slabs; 1/denom commutes through
    the value matmul, so mm2 consumes W directly (contiguous PE transposes,
    no dependency on the cross-half denominator)
  - A_tild = W * (0.5/denom) interleaved for DRAM, off the mm2 critical path
  - V_att = psum(W^T.V) * (0.5/denom) * log1p(degrees) at the end (single Ln)
"""

import sys

if "/opt/trn_rl_repo" not in sys.path:
    sys.path.insert(0, "/opt/trn_rl_repo")

import contextlib
from contextlib import ExitStack

import numpy as np

import concourse.bass as bass
import concourse.tile as tile
from concourse import bacc, masks, mybir
from concourse._compat import with_exitstack
from concourse.bass_utils import run_bass_kernel_spmd

F32 = mybir.dt.float32
AF = mybir.ActivationFunctionType
ALU = mybir.AluOpType

B, N, H, D = 4, 1024, 8, 32
P = 128            # sbuf partitions / l-tile rows
LCORE = 512        # l rows per core
NLT = LCORE // P   # l-tiles per core (4)
MH = 512           # m-half size
NMH = N // MH      # m halves (2)
NCH = MH // P      # 128-chunks per m half (4)
NT = N // P        # m tiles over full N (8)
CLIP = 5.0


@with_exitstack
def egt_body(ctx: ExitStack, tc: tile.TileContext, outs, ins, repeat=1,
             alias_rows=False, variant=()):
    nc = tc.nc
    h_out, a_out, v_out = outs
    qkv, q_in, e_in, g_in = ins

    def rows(i):
        # timing-alias mode: all l-tiles hit the same 128 DRAM rows so the
        # timing build can use tiny I/O tensors (identical instruction stream)
        return slice(0, P) if alias_rows else slice(i * P, (i + 1) * P)

    rep_ctx = tc.For_i(0, repeat, 1) if repeat > 1 else None

    const = ctx.enter_context(tc.tile_pool(name="const", bufs=1))
    setup = ctx.enter_context(tc.tile_pool(name="setup", bufs=2))
    hpool = ctx.enter_context(tc.tile_pool(name="hpool", bufs=2))
    gpool = ctx.enter_context(tc.tile_pool(name="gpool", bufs=2))
    apool = ctx.enter_context(tc.tile_pool(name="apool", bufs=2))
    wpool = ctx.enter_context(tc.tile_pool(name="wpool", bufs=3))
    upool = ctx.enter_context(tc.tile_pool(name="upool", bufs=3))
    atpool = ctx.enter_context(tc.tile_pool(name="atpool", bufs=2))
    smalls = ctx.enter_context(tc.tile_pool(name="smalls", bufs=1))
    ps_mm1 = ctx.enter_context(tc.tile_pool(name="ps_mm1", bufs=2, space="PSUM"))
    ps_tr = ctx.enter_context(tc.tile_pool(name="ps_tr", bufs=2, space="PSUM"))
    ps_v = ctx.enter_context(tc.tile_pool(name="ps_v", bufs=2, space="PSUM"))

    ident = const.tile([P, P], F32)
    masks.make_identity(nc, ident[:])
    ln_bias = const.tile([P, 1], F32)
    nc.gpsimd.memset(ln_bias[:], 1.0 + N / 2.0)

    with rep_ctx if rep_ctx is not None else contextlib.nullcontext():
        # ---- setup: V native, K^T / Q^T per head ----
        v_sb = const.tile([P, NT * D * H], F32, tag="v_sb")
        # packed transposed K/Q: partition 32*g + d holds head h = 2*g + hh
        k_t = const.tile([P, 2 * N], F32, tag="k_t")      # [32g+d, hh*N + m]
        q_t = const.tile([P, 2 * LCORE], F32, tag="q_t")  # [32g+d, hh*LCORE + l]

        for t in range(NT):
            kv_nat = setup.tile([P, 2 * D * H], F32, tag="kv_nat")
            nc.sync.dma_start(kv_nat[:], qkv[t * P : (t + 1) * P, D * H : 3 * D * H])
            nc.vector.tensor_copy(
                v_sb[:, t * D * H : (t + 1) * D * H], kv_nat[:, D * H : 2 * D * H]
            )
            # one [128,128] transpose per hh gathers heads {hh, hh+2, hh+4,
            # hh+6}: out partition 32*g + d <- column d*H + (2g + hh)
            kv_v = kv_nat[:, 0 : D * H].rearrange("p (d h) -> p h d", h=H)
            pst = ps_tr.tile([P, 2 * P], F32, tag="pst")
            stage = setup.tile([P, 2 * P], F32, tag="stage")
            for hh in range(2):
                nc.vector.tensor_copy(
                    stage[:, hh * P : (hh + 1) * P], kv_v[:, hh::2, :]
                )
                nc.tensor.transpose(
                    pst[:, hh * P : (hh + 1) * P],
                    stage[:, hh * P : (hh + 1) * P],
                    ident[:],
                )
            nc.scalar.copy(
                k_t[:].rearrange("p (hh m) -> p hh m", hh=2)[:, :, t * P : (t + 1) * P],
                pst[:].rearrange("p (hh c) -> p hh c", hh=2),
            )

        for i in range(NLT):
            q_nat = setup.tile([P, D * H], F32, tag="q_nat")
            nc.sync.dma_start(q_nat[:], q_in[rows(i), :])
            q_v = q_nat[:].rearrange("p (d h) -> p h d", h=H)
            pst = ps_tr.tile([P, 2 * P], F32, tag="pst")
            stage = setup.tile([P, 2 * P], F32, tag="stage")
            for hh in range(2):
                nc.vector.tensor_copy(
                    stage[:, hh * P : (hh + 1) * P], q_v[:, hh::2, :]
                )
                nc.tensor.transpose(
                    pst[:, hh * P : (hh + 1) * P],
                    stage[:, hh * P : (hh + 1) * P],
                    ident[:],
                )
            nc.scalar.copy(
                q_t[:].rearrange("p (hh l) -> p hh l", hh=2)[
                    :, :, i * P : (i + 1) * P
                ],
                pst[:].rearrange("p (hh c) -> p hh c", hh=2),
            )

        # per-core running stats
        sth_all = smalls.tile([P, NLT * H], F32, tag="sth")     # sum tanh
        hr_all = smalls.tile([P, NLT * H], F32, tag="hr_all")   # 0.5/denom
        vraw = smalls.tile([P, NLT * D * H], F32, tag="vraw")   # unscaled V_att

        for i in range(NLT):
            den = smalls.tile([P, NMH * H], F32, tag="den")
            sth = smalls.tile([P, NMH * H], F32, tag="sthp")
            psv = ps_v.tile([P, H * D], F32, tag="psv")
            w_tiles = []
            for s in range(NMH):
                hsb = hpool.tile([P, MH * H], F32, tag="hsb")
                gsb = gpool.tile([P, MH * H], F32, tag="gsb")
                wf = wpool.tile([P, MH * H], F32, tag="wf")   # exp -> W slabs
                uth0 = upool.tile([P, 4 * MH], F32, tag="uth")
                uth1 = upool.tile([P, 4 * MH], F32, tag="uth")
                uths = [uth0, uth1]
                w_tiles.append(wf)
                nc.sync.dma_start(
                    gsb[:], g_in[rows(i), s * MH * H : (s + 1) * MH * H]
                )
                # mm1 (head pairs share a 2-bank psum tile) + wide 2-op clip
                # into the interleaved layout via a 3D output view
                hsb_r = hsb[:].rearrange("p (m h) -> p h m", h=H)
                for g in range(4):
                    mm = ps_mm1.tile([P, 2 * MH], F32, tag="mm1")
                    for hh in range(2):
                        nc.tensor.matmul(
                            mm[:, hh * MH : (hh + 1) * MH],
                            q_t[
                                D * g : D * (g + 1),
                                hh * LCORE + i * P : hh * LCORE + (i + 1) * P,
                            ],
                            k_t[
                                D * g : D * (g + 1),
                                hh * N + s * MH : hh * N + (s + 1) * MH,
                            ],
                            start=True,
                            stop=True,
                            tile_position=(D * g, 0),
                        )
                    nc.vector.tensor_scalar(
                        out=hsb_r[:, 2 * g : 2 * g + 2, :],
                        in0=mm[:],
                        scalar1=CLIP,
                        scalar2=-CLIP,
                        op0=ALU.min,
                        op1=ALU.max,
                    )
                # H_hat = clip + E via accumulate DMA (CCE adder caps at 2048
                # elements per descriptor row -> chunk)
                if "no_accum" not in variant:
                    for o in range(0, MH * H, 2048):
                        nc.gpsimd.dma_start(
                            hsb[:, o : o + 2048],
                            e_in[rows(i), s * MH * H + o : s * MH * H + o + 2048],
                            accum_op=ALU.add,
                        )
                if "no_hout" not in variant:
                    nc.sync.dma_start(
                        h_out[rows(i), s * MH * H : (s + 1) * MH * H], hsb[:]
                    )
                # per-head exp/tanh: strided-in -> flat slab out, fused
                # accum_out row sums (denominator and degrees for free)
                for h in range(H):
                    nc.scalar.activation(
                        wf[:, h * MH : (h + 1) * MH],
                        hsb[:, h::H],
                        AF.Exp,
                        accum_out=den[:, s * H + h : s * H + h + 1],
                    )
                    nc.scalar.activation(
                        uths[h // 4][:, (h % 4) * MH : (h % 4 + 1) * MH],
                        gsb[:, h::H],
                        AF.Tanh,
                        scale=0.5,
                        accum_out=sth[:, s * H + h : s * H + h + 1],
                    )
                # W = exp*(1+tanh) = exp + exp*tanh, built on gpsimd in
                # half-tile chunks (W lands in-place in wf; uth is scratch)
                if "no_gpsw" not in variant:
                    for q in range(2):
                        cs = slice(q * 4 * MH, (q + 1) * 4 * MH)
                        u = uths[q]
                        nc.gpsimd.tensor_tensor(u[:], u[:], wf[:, cs], ALU.mult)
                        nc.gpsimd.tensor_tensor(wf[:, cs], wf[:, cs], u[:], ALU.add)
                # transposes of flat W chunks + mm2 (no denom dependency)
                for h in (() if "no_pe2" in variant else range(H)):
                    pst = ps_tr.tile([P, NCH * P], F32, tag="pst")
                    for c in range(NCH):
                        nc.tensor.transpose(
                            pst[:, c * P : (c + 1) * P],
                            wf[:, h * MH + c * P : h * MH + (c + 1) * P],
                            ident[:],
                        )
                    at_sb = atpool.tile([P, NCH * P], F32, tag="at_sb")
                    nc.any.tensor_copy(at_sb[:], pst[:])
                    for c in range(NCH):
                        t = s * NCH + c  # global m tile index
                        # one accumulation group spans the whole psv bank
                        nc.tensor.matmul(
                            psv[:, h * D : (h + 1) * D],
                            at_sb[:, c * P : (c + 1) * P],
                            v_sb[:, t * D * H + h : (t + 1) * D * H : H],
                            start=(s == 0 and h == 0 and c == 0),
                            stop=(s == NMH - 1 and h == H - 1 and c == NCH - 1),
                        )

            # 0.5/denom over both halves
            denom = smalls.tile([P, H], F32, tag="denom")
            nc.vector.tensor_add(denom[:], den[:, 0:H], den[:, H : 2 * H])
            nc.vector.tensor_add(
                sth_all[:, i * H : (i + 1) * H], sth[:, 0:H], sth[:, H : 2 * H]
            )
            half_r = hr_all[:, i * H : (i + 1) * H]
            nc.vector.reciprocal(half_r, denom[:])
            nc.vector.tensor_scalar_mul(half_r, half_r, 0.5)

            # A_tild = W * (0.5/denom), interleaved for DRAM ([128,2048] chunks)
            QW = MH * H // 2
            for s in (() if "no_aout" in variant else range(NMH)):
                wf = w_tiles[s]
                for q in range(2):
                    asb = apool.tile([P, QW], F32, tag="asb")
                    mq = MH // 2  # m-columns per chunk
                    for h in range(H):
                        nc.any.tensor_scalar(
                            out=asb[:, h::H],
                            in0=wf[:, h * MH + q * mq : h * MH + (q + 1) * mq],
                            scalar1=half_r[:, h : h + 1],
                            scalar2=None,
                            op0=ALU.mult,
                        )
                    nc.sync.dma_start(
                        a_out[rows(i), s * MH * H + q * QW : s * MH * H + (q + 1) * QW],
                        asb[:],
                    )
            # evict raw V_att (scaled at the end)
            if "no_pe2" in variant:
                nc.vector.tensor_copy(
                    vraw[:, i * D * H : (i + 1) * D * H], w_tiles[0][:, 0 : D * H]
                )
            else:
                nc.scalar.copy(vraw[:, i * D * H : (i + 1) * D * H], psv[:])

        # ---- final: V_att = vraw * (0.5/denom) * log1p(N/2 + 0.5*sum tanh) ----
        lnsc = smalls.tile([P, NLT * H], F32, tag="lnsc")
        nc.scalar.activation(lnsc[:], sth_all[:], AF.Ln, scale=0.5, bias=ln_bias[:])
        nc.vector.tensor_mul(lnsc[:], lnsc[:], hr_all[:])
        vout_sb = smalls.tile([P, NLT * D * H], F32, tag="vout_sb")
        for i in range(NLT):
            for h in range(H):
                nc.vector.tensor_scalar(
                    out=vout_sb[:, i * D * H + h : (i + 1) * D * H : H],
                    in0=vraw[:, i * D * H + h * D : i * D * H + (h + 1) * D],
                    scalar1=lnsc[:, i * H + h : i * H + h + 1],
                    scalar2=None,
                    op0=ALU.mult,
                )
            nc.sync.dma_start(
                v_out[rows(i), :], vout_sb[:, i * D * H : (i + 1) * D * H]
            )


_programs = {}


def build_program(repeat=1, alias_rows=False, variant=()):
    key = (repeat, alias_rows, tuple(sorted(variant)))
    if key in _programs:
        return _programs[key]
    RR = P if alias_rows else LCORE
    nc = bacc.Bacc("TRN2", target_bir_lowering=False, debug=False, num_devices=8)
    qkv = nc.dram_tensor("qkv", [N, 3 * D * H], F32, kind="ExternalInput").ap()
    q_in = nc.dram_tensor("q_in", [RR, D * H], F32, kind="ExternalInput").ap()
    e_in = nc.dram_tensor("e_in", [RR, N * H], F32, kind="ExternalInput").ap()
    g_in = nc.dram_tensor("g_in", [RR, N * H], F32, kind="ExternalInput").ap()
    h_out = nc.dram_tensor("h_out", [RR, N * H], F32, kind="ExternalOutput").ap()
    a_out = nc.dram_tensor("a_out", [RR, N * H], F32, kind="ExternalOutput").ap()
    v_out = nc.dram_tensor("v_out", [RR, D * H], F32, kind="ExternalOutput").ap()
    with tile.TileContext(nc) as tc:
        egt_body(tc, (h_out, a_out, v_out), (qkv, q_in, e_in, g_in), repeat=repeat,
                 alias_rows=alias_rows, variant=variant)
    nc.compile()
    _programs[key] = nc
    return nc


def kernel(QKV, E, G, repeat=1, _timing_out=None):
    QKV = np.ascontiguousarray(np.asarray(QKV, dtype=np.float32))
    E = np.ascontiguousarray(np.asarray(E, dtype=np.float32))
    G = np.ascontiguousarray(np.asarray(G, dtype=np.float32))
    assert QKV.shape == (B, N, 3 * D * H)
    assert E.shape == (B, N, N, H) and G.shape == (B, N, N, H)

    nc = build_program(repeat)
    in_maps = []
    for c in range(8):
        b, lh = c // 2, c % 2
        sl = slice(lh * LCORE, (lh + 1) * LCORE)
        in_maps.append(
            {
                "qkv": QKV[b],
                "q_in": np.ascontiguousarray(QKV[b, sl, 0 : D * H]),
                "e_in": np.ascontiguousarray(E[b, sl].reshape(LCORE, N * H)),
                "g_in": np.ascontiguousarray(G[b, sl].reshape(LCORE, N * H)),
            }
        )
    import time

    t0 = time.perf_counter()
    res = run_bass_kernel_spmd(nc, in_maps, list(range(8))).results
    t1 = time.perf_counter()
    if _timing_out is not None:
        _timing_out.append(t1 - t0)

    V_att = np.empty((B, N, D * H), np.float32)
    H_hat = np.empty((B, N, N, H), np.float32)
    A_tild = np.empty((B, N, N, H), np.float32)
    for c, r in enumerate(res):
        b, lh = c // 2, c % 2
        sl = slice(lh * LCORE, (lh + 1) * LCORE)
        V_att[b, sl] = r["v_out"]
        H_hat[b, sl] = r["h_out"].reshape(LCORE, N, H)
        A_tild[b, sl] = r["a_out"].reshape(LCORE, N, H)
    return V_att, H_hat, A_tild


# revision 31
# speedup vs baseline: 1.2081x; 1.0099x over previous
"""EGT (edge-gated transformer) layer as a Bass/Tile kernel on 8 trn2 cores.

Problem (hardcoded shapes): B=4, N=1024, H=8, D=32.
  QKV [4,1024,768], E [4,1024,1024,8], G [4,1024,1024,8]
  returns (V_att [4,1024,256], H_hat [4,1024,1024,8], A_tild [4,1024,1024,8])

Sharding: core c handles batch b=c//2 and query-row half l0=(c%2)*512.
Each core reads QKV[b] (3MB) + E,G row-slices (16MB each) and writes
H_hat, A_tild row-slices (16MB each) + V_att slice (0.5MB).

Per-core dataflow (l-tiles of 128 rows x m-halves of 512 keys):
  - setup: load V natively; build K^T,Q^T ([d, m|l] per head) via PE transposes
  - mm1: A = Q.K^T per head (32-partition contraction)
  - clip via one 2-op tensor_scalar (min,max) PSUM->SBUF, h-interleaved layout
  - H_hat = clip(A) + E with E added by a gpsimd accumulate-DMA (CCE-chunked)
  - exp/tanh on ACT (strided in, flat slab out) with fused accum_out row sums;
    sigmoid(g) == (1+tanh(g/2))/2 keeps everything in one ACT table set
  - W = exp*(1+tanh) kept in flat per-head slabs; 1/denom commutes through
    the value matmul, so mm2 consumes W directly (contiguous PE transposes,
    no dependency on the cross-half denominator)
  - A_tild = W * (0.5/denom) interleaved for DRAM, off the mm2 critical path
  - V_att = psum(W^T.V) * (0.5/denom) * log1p(degrees) at the end (single Ln)
"""

import sys

if "/opt/trn_rl_repo" not in sys.path:
    sys.path.insert(0, "/opt/trn_rl_repo")

import contextlib
from contextlib import ExitStack

import numpy as np

import concourse.bass as bass
import concourse.tile as tile
from concourse import bacc, masks, mybir
from concourse._compat import with_exitstack
from concourse.bass_utils import run_bass_kernel_spmd

F32 = mybir.dt.float32
AF = mybir.ActivationFunctionType
ALU = mybir.AluOpType

B, N, H, D = 4, 1024, 8, 32
P = 128            # sbuf partitions / l-tile rows
LCORE = 512        # l rows per core
NLT = LCORE // P   # l-tiles per core (4)
MH = 512           # m-half size
NMH = N // MH      # m halves (2)
NCH = MH // P      # 128-chunks per m half (4)
NT = N // P        # m tiles over full N (8)
CLIP = 5.0


@with_exitstack
def egt_body(ctx: ExitStack, tc: tile.TileContext, outs, ins, repeat=1,
             alias_rows=False, variant=()):
    nc = tc.nc
    h_out, a_out, v_out = outs
    qkv, q_in, e_in, g_in = ins

    def rows(i):
        # timing-alias mode: all l-tiles hit the same 128 DRAM rows so the
        # timing build can use tiny I/O tensors (identical instruction stream)
        return slice(0, P) if alias_rows else slice(i * P, (i + 1) * P)

    rep_ctx = tc.For_i(0, repeat, 1) if repeat > 1 else None

    const = ctx.enter_context(tc.tile_pool(name="const", bufs=1))
    setup = ctx.enter_context(tc.tile_pool(name="setup", bufs=2))
    hpool = ctx.enter_context(tc.tile_pool(name="hpool", bufs=2))
    gpool = ctx.enter_context(tc.tile_pool(name="gpool", bufs=2))
    apool = ctx.enter_context(tc.tile_pool(name="apool", bufs=2))
    wpool = ctx.enter_context(tc.tile_pool(name="wpool", bufs=3))
    upool = ctx.enter_context(tc.tile_pool(name="upool", bufs=3))
    atpool = ctx.enter_context(tc.tile_pool(name="atpool", bufs=4))
    smalls = ctx.enter_context(tc.tile_pool(name="smalls", bufs=1))
    ps_mm1 = ctx.enter_context(tc.tile_pool(name="ps_mm1", bufs=2, space="PSUM"))
    ps_tr = ctx.enter_context(tc.tile_pool(name="ps_tr", bufs=2, space="PSUM"))
    ps_v = ctx.enter_context(tc.tile_pool(name="ps_v", bufs=2, space="PSUM"))

    ident = const.tile([P, P], F32)
    masks.make_identity(nc, ident[:])
    ln_bias = const.tile([P, 1], F32)
    nc.gpsimd.memset(ln_bias[:], 1.0 + N / 2.0)

    with rep_ctx if rep_ctx is not None else contextlib.nullcontext():
        # ---- setup: V native, K^T / Q^T per head ----
        v_sb = const.tile([P, NT * D * H], F32, tag="v_sb")
        # packed transposed K/Q: partition 32*g + d holds head h = 2*g + hh
        k_t = const.tile([P, 2 * N], F32, tag="k_t")      # [32g+d, hh*N + m]
        q_t = const.tile([P, 2 * LCORE], F32, tag="q_t")  # [32g+d, hh*LCORE + l]

        for t in range(NT):
            kv_nat = setup.tile([P, 2 * D * H], F32, tag="kv_nat")
            nc.sync.dma_start(kv_nat[:], qkv[t * P : (t + 1) * P, D * H : 3 * D * H])
            nc.vector.tensor_copy(
                v_sb[:, t * D * H : (t + 1) * D * H], kv_nat[:, D * H : 2 * D * H]
            )
            # one [128,128] transpose per hh gathers heads {hh, hh+2, hh+4,
            # hh+6}: out partition 32*g + d <- column d*H + (2g + hh)
            kv_v = kv_nat[:, 0 : D * H].rearrange("p (d h) -> p h d", h=H)
            pst = ps_tr.tile([P, 2 * P], F32, tag="pst")
            stage = setup.tile([P, 2 * P], F32, tag="stage")
            for hh in range(2):
                nc.vector.tensor_copy(
                    stage[:, hh * P : (hh + 1) * P], kv_v[:, hh::2, :]
                )
                nc.tensor.transpose(
                    pst[:, hh * P : (hh + 1) * P],
                    stage[:, hh * P : (hh + 1) * P],
                    ident[:],
                )
            nc.scalar.copy(
                k_t[:].rearrange("p (hh m) -> p hh m", hh=2)[:, :, t * P : (t + 1) * P],
                pst[:].rearrange("p (hh c) -> p hh c", hh=2),
            )

        for i in range(NLT):
            q_nat = setup.tile([P, D * H], F32, tag="q_nat")
            nc.sync.dma_start(q_nat[:], q_in[rows(i), :])
            q_v = q_nat[:].rearrange("p (d h) -> p h d", h=H)
            pst = ps_tr.tile([P, 2 * P], F32, tag="pst")
            stage = setup.tile([P, 2 * P], F32, tag="stage")
            for hh in range(2):
                nc.vector.tensor_copy(
                    stage[:, hh * P : (hh + 1) * P], q_v[:, hh::2, :]
                )
                nc.tensor.transpose(
                    pst[:, hh * P : (hh + 1) * P],
                    stage[:, hh * P : (hh + 1) * P],
                    ident[:],
                )
            nc.scalar.copy(
                q_t[:].rearrange("p (hh l) -> p hh l", hh=2)[
                    :, :, i * P : (i + 1) * P
                ],
                pst[:].rearrange("p (hh c) -> p hh c", hh=2),
            )

        # per-core running stats
        sth_all = smalls.tile([P, NLT * H], F32, tag="sth")     # sum tanh
        hr_all = smalls.tile([P, NLT * H], F32, tag="hr_all")   # 0.5/denom
        vraw = smalls.tile([P, NLT * D * H], F32, tag="vraw")   # unscaled V_att

        for i in range(NLT):
            den = smalls.tile([P, NMH * H], F32, tag="den")
            sth = smalls.tile([P, NMH * H], F32, tag="sthp")
            psv = ps_v.tile([P, H * D], F32, tag="psv")
            w_tiles = []
            for s in range(NMH):
                hsb = hpool.tile([P, MH * H], F32, tag="hsb")
                gsb = gpool.tile([P, MH * H], F32, tag="gsb")
                wf = wpool.tile([P, MH * H], F32, tag="wf")   # exp -> W slabs
                uth0 = upool.tile([P, 4 * MH], F32, tag="uth")
                uth1 = upool.tile([P, 4 * MH], F32, tag="uth")
                uths = [uth0, uth1]
                w_tiles.append(wf)
                nc.sync.dma_start(
                    gsb[:], g_in[rows(i), s * MH * H : (s + 1) * MH * H]
                )
                # mm1 (head pairs share a 2-bank psum tile) + wide 2-op clip
                # into the interleaved layout via a 3D output view
                hsb_r = hsb[:].rearrange("p (m h) -> p h m", h=H)
                for g in range(4):
                    mm = ps_mm1.tile([P, 2 * MH], F32, tag="mm1")
                    for hh in range(2):
                        nc.tensor.matmul(
                            mm[:, hh * MH : (hh + 1) * MH],
                            q_t[
                                D * g : D * (g + 1),
                                hh * LCORE + i * P : hh * LCORE + (i + 1) * P,
                            ],
                            k_t[
                                D * g : D * (g + 1),
                                hh * N + s * MH : hh * N + (s + 1) * MH,
                            ],
                            start=True,
                            stop=True,
                            tile_position=(D * g, 0),
                        )
                    nc.vector.tensor_scalar(
                        out=hsb_r[:, 2 * g : 2 * g + 2, :],
                        in0=mm[:],
                        scalar1=CLIP,
                        scalar2=-CLIP,
                        op0=ALU.min,
                        op1=ALU.max,
                    )
                # H_hat = clip + E via accumulate DMA (CCE adder caps at 2048
                # elements per descriptor row -> chunk)
                if "no_accum" not in variant:
                    for o in range(0, MH * H, 2048):
                        nc.gpsimd.dma_start(
                            hsb[:, o : o + 2048],
                            e_in[rows(i), s * MH * H + o : s * MH * H + o + 2048],
                            accum_op=ALU.add,
                        )
                if "no_hout" not in variant:
                    nc.sync.dma_start(
                        h_out[rows(i), s * MH * H : (s + 1) * MH * H], hsb[:]
                    )
                # per-head exp/tanh: strided-in -> flat slab out, fused
                # accum_out row sums (denominator and degrees for free)
                for h in range(H):
                    nc.scalar.activation(
                        wf[:, h * MH : (h + 1) * MH],
                        hsb[:, h::H],
                        AF.Exp,
                        accum_out=den[:, s * H + h : s * H + h + 1],
                    )
                    nc.scalar.activation(
                        uths[h // 4][:, (h % 4) * MH : (h % 4 + 1) * MH],
                        gsb[:, h::H],
                        AF.Tanh,
                        scale=0.5,
                        accum_out=sth[:, s * H + h : s * H + h + 1],
                    )
                # W = exp*(1+tanh) = exp + exp*tanh, built on gpsimd in
                # half-tile chunks (W lands in-place in wf; uth is scratch)
                if "no_gpsw" not in variant:
                    for q in range(2):
                        cs = slice(q * 4 * MH, (q + 1) * 4 * MH)
                        u = uths[q]
                        nc.gpsimd.tensor_tensor(u[:], u[:], wf[:, cs], ALU.mult)
                        nc.gpsimd.tensor_tensor(wf[:, cs], wf[:, cs], u[:], ALU.add)
                # transposes of flat W chunks + mm2 (no denom dependency)
                for h in (() if "no_pe2" in variant else range(H)):
                    pst = ps_tr.tile([P, NCH * P], F32, tag="pst")
                    for c in range(NCH):
                        nc.tensor.transpose(
                            pst[:, c * P : (c + 1) * P],
                            wf[:, h * MH + c * P : h * MH + (c + 1) * P],
                            ident[:],
                        )
                    at_sb = atpool.tile([P, NCH * P], F32, tag="at_sb")
                    nc.any.tensor_copy(at_sb[:], pst[:])
                    for c in range(NCH):
                        t = s * NCH + c  # global m tile index
                        # one accumulation group spans the whole psv bank
                        nc.tensor.matmul(
                            psv[:, h * D : (h + 1) * D],
                            at_sb[:, c * P : (c + 1) * P],
                            v_sb[:, t * D * H + h : (t + 1) * D * H : H],
                            start=(s == 0 and h == 0 and c == 0),
                            stop=(s == NMH - 1 and h == H - 1 and c == NCH - 1),
                        )

            # 0.5/denom over both halves
            denom = smalls.tile([P, H], F32, tag="denom")
            nc.vector.tensor_add(denom[:], den[:, 0:H], den[:, H : 2 * H])
            nc.vector.tensor_add(
                sth_all[:, i * H : (i + 1) * H], sth[:, 0:H], sth[:, H : 2 * H]
            )
            half_r = hr_all[:, i * H : (i + 1) * H]
            nc.vector.reciprocal(half_r, denom[:])
            nc.vector.tensor_scalar_mul(half_r, half_r, 0.5)

            # A_tild = W * (0.5/denom), interleaved for DRAM ([128,2048] chunks)
            QW = MH * H // 2
            for s in (() if "no_aout" in variant else range(NMH)):
                wf = w_tiles[s]
                for q in range(2):
                    asb = apool.tile([P, QW], F32, tag="asb")
                    mq = MH // 2  # m-columns per chunk
                    for h in range(H):
                        nc.any.tensor_scalar(
                            out=asb[:, h::H],
                            in0=wf[:, h * MH + q * mq : h * MH + (q + 1) * mq],
                            scalar1=half_r[:, h : h + 1],
                            scalar2=None,
                            op0=ALU.mult,
                        )
                    nc.sync.dma_start(
                        a_out[rows(i), s * MH * H + q * QW : s * MH * H + (q + 1) * QW],
                        asb[:],
                    )
            # evict raw V_att (scaled at the end)
            if "no_pe2" in variant:
                nc.vector.tensor_copy(
                    vraw[:, i * D * H : (i + 1) * D * H], w_tiles[0][:, 0 : D * H]
                )
            else:
                nc.scalar.copy(vraw[:, i * D * H : (i + 1) * D * H], psv[:])

        # ---- final: V_att = vraw * (0.5/denom) * log1p(N/2 + 0.5*sum tanh) ----
        lnsc = smalls.tile([P, NLT * H], F32, tag="lnsc")
        nc.scalar.activation(lnsc[:], sth_all[:], AF.Ln, scale=0.5, bias=ln_bias[:])
        nc.vector.tensor_mul(lnsc[:], lnsc[:], hr_all[:])
        vout_sb = smalls.tile([P, NLT * D * H], F32, tag="vout_sb")
        for i in range(NLT):
            for h in range(H):
                nc.vector.tensor_scalar(
                    out=vout_sb[:, i * D * H + h : (i + 1) * D * H : H],
                    in0=vraw[:, i * D * H + h * D : i * D * H + (h + 1) * D],
                    scalar1=lnsc[:, i * H + h : i * H + h + 1],
                    scalar2=None,
                    op0=ALU.mult,
                )
            nc.sync.dma_start(
                v_out[rows(i), :], vout_sb[:, i * D * H : (i + 1) * D * H]
            )


_programs = {}


def build_program(repeat=1, alias_rows=False, variant=()):
    key = (repeat, alias_rows, tuple(sorted(variant)))
    if key in _programs:
        return _programs[key]
    RR = P if alias_rows else LCORE
    nc = bacc.Bacc("TRN2", target_bir_lowering=False, debug=False, num_devices=8)
    qkv = nc.dram_tensor("qkv", [N, 3 * D * H], F32, kind="ExternalInput").ap()
    q_in = nc.dram_tensor("q_in", [RR, D * H], F32, kind="ExternalInput").ap()
    e_in = nc.dram_tensor("e_in", [RR, N * H], F32, kind="ExternalInput").ap()
    g_in = nc.dram_tensor("g_in", [RR, N * H], F32, kind="ExternalInput").ap()
    h_out = nc.dram_tensor("h_out", [RR, N * H], F32, kind="ExternalOutput").ap()
    a_out = nc.dram_tensor("a_out", [RR, N * H], F32, kind="ExternalOutput").ap()
    v_out = nc.dram_tensor("v_out", [RR, D * H], F32, kind="ExternalOutput").ap()
    with tile.TileContext(nc) as tc:
        egt_body(tc, (h_out, a_out, v_out), (qkv, q_in, e_in, g_in), repeat=repeat,
                 alias_rows=alias_rows, variant=variant)
    nc.compile()
    _programs[key] = nc
    return nc


def kernel(QKV, E, G, repeat=1, _timing_out=None):
    QKV = np.ascontiguousarray(np.asarray(QKV, dtype=np.float32))
    E = np.ascontiguousarray(np.asarray(E, dtype=np.float32))
    G = np.ascontiguousarray(np.asarray(G, dtype=np.float32))
    assert QKV.shape == (B, N, 3 * D * H)
    assert E.shape == (B, N, N, H) and G.shape == (B, N, N, H)

    nc = build_program(repeat)
    in_maps = []
    for c in range(8):
        b, lh = c // 2, c % 2
        sl = slice(lh * LCORE, (lh + 1) * LCORE)
        in_maps.append(
            {
                "qkv": QKV[b],
                "q_in": np.ascontiguousarray(QKV[b, sl, 0 : D * H]),
                "e_in": np.ascontiguousarray(E[b, sl].reshape(LCORE, N * H)),
                "g_in": np.ascontiguousarray(G[b, sl].reshape(LCORE, N * H)),
            }
        )
    import time

    t0 = time.perf_counter()
    res = run_bass_kernel_spmd(nc, in_maps, list(range(8))).results
    t1 = time.perf_counter()
    if _timing_out is not None:
        _timing_out.append(t1 - t0)

    V_att = np.empty((B, N, D * H), np.float32)
    H_hat = np.empty((B, N, N, H), np.float32)
    A_tild = np.empty((B, N, N, H), np.float32)
    for c, r in enumerate(res):
        b, lh = c // 2, c % 2
        sl = slice(lh * LCORE, (lh + 1) * LCORE)
        V_att[b, sl] = r["v_out"]
        H_hat[b, sl] = r["h_out"].reshape(LCORE, N, H)
        A_tild[b, sl] = r["a_out"].reshape(LCORE, N, H)
    return V_att, H_hat, A_tild


# revision 33
# speedup vs baseline: 1.2615x; 1.0442x over previous
"""EGT (edge-gated transformer) layer as a Bass/Tile kernel on 8 trn2 cores.

Problem (hardcoded shapes): B=4, N=1024, H=8, D=32.
  QKV [4,1024,768], E [4,1024,1024,8], G [4,1024,1024,8]
  returns (V_att [4,1024,256], H_hat [4,1024,1024,8], A_tild [4,1024,1024,8])

Sharding: core c handles batch b=c//2 and query-row half l0=(c%2)*512.
Each core reads QKV[b] (3MB) + E,G row-slices (16MB each) and writes
H_hat, A_tild row-slices (16MB each) + V_att slice (0.5MB).

Per-core dataflow (l-tiles of 128 rows x m-halves of 512 keys):
  - setup: load V natively; build K^T,Q^T ([d, m|l] per head) via PE transposes
  - mm1: A = Q.K^T per head (32-partition contraction)
  - clip via one 2-op tensor_scalar (min,max) PSUM->SBUF, h-interleaved layout
  - H_hat = clip(A) + E with E added by a gpsimd accumulate-DMA (CCE-chunked)
  - exp/tanh on ACT (strided in, flat slab out) with fused accum_out row sums;
    sigmoid(g) == (1+tanh(g/2))/2 keeps everything in one ACT table set
  - W = exp*(1+tanh) kept in flat per-head slabs; 1/denom commutes through
    the value matmul, so mm2 consumes W directly (contiguous PE transposes,
    no dependency on the cross-half denominator)
  - A_tild = W * (0.5/denom) interleaved for DRAM, off the mm2 critical path
  - V_att = psum(W^T.V) * (0.5/denom) * log1p(degrees) at the end (single Ln)
"""

import sys

if "/opt/trn_rl_repo" not in sys.path:
    sys.path.insert(0, "/opt/trn_rl_repo")

import contextlib
from contextlib import ExitStack

import numpy as np

import concourse.bass as bass
import concourse.tile as tile
from concourse import bacc, masks, mybir
from concourse._compat import with_exitstack
from concourse.bass_utils import run_bass_kernel_spmd

F32 = mybir.dt.float32
AF = mybir.ActivationFunctionType
ALU = mybir.AluOpType

B, N, H, D = 4, 1024, 8, 32
P = 128            # sbuf partitions / l-tile rows
LCORE = 512        # l rows per core
NLT = LCORE // P   # l-tiles per core (4)
MH = 512           # m-half size
NMH = N // MH      # m halves (2)
NCH = MH // P      # 128-chunks per m half (4)
NT = N // P        # m tiles over full N (8)
CLIP = 5.0


@with_exitstack
def egt_body(ctx: ExitStack, tc: tile.TileContext, outs, ins, repeat=1,
             alias_rows=False, variant=()):
    nc = tc.nc
    h_out, a_out, v_out = outs
    qkv, q_in, e_in, g_in = ins

    def rows(i):
        # timing-alias mode: all l-tiles hit the same 128 DRAM rows so the
        # timing build can use tiny I/O tensors (identical instruction stream)
        return slice(0, P) if alias_rows else slice(i * P, (i + 1) * P)

    rep_ctx = tc.For_i(0, repeat, 1) if repeat > 1 else None

    const = ctx.enter_context(tc.tile_pool(name="const", bufs=1))
    setup = ctx.enter_context(tc.tile_pool(name="setup", bufs=2))
    hpool = ctx.enter_context(tc.tile_pool(name="hpool", bufs=2))
    gpool = ctx.enter_context(tc.tile_pool(name="gpool", bufs=2))
    apool = ctx.enter_context(tc.tile_pool(name="apool", bufs=2))
    wpool = ctx.enter_context(tc.tile_pool(name="wpool", bufs=3))
    upool = ctx.enter_context(tc.tile_pool(name="upool", bufs=3))
    atpool = ctx.enter_context(tc.tile_pool(name="atpool", bufs=4))
    smalls = ctx.enter_context(tc.tile_pool(name="smalls", bufs=1))
    ps_mm1 = ctx.enter_context(tc.tile_pool(name="ps_mm1", bufs=2, space="PSUM"))
    ps_tr = ctx.enter_context(tc.tile_pool(name="ps_tr", bufs=2, space="PSUM"))
    ps_v = ctx.enter_context(tc.tile_pool(name="ps_v", bufs=2, space="PSUM"))

    ident = const.tile([P, P], F32)
    masks.make_identity(nc, ident[:])
    ln_bias = const.tile([P, 1], F32)
    nc.gpsimd.memset(ln_bias[:], 1.0 + N / 2.0)

    with rep_ctx if rep_ctx is not None else contextlib.nullcontext():
        # ---- setup: V native, K^T / Q^T per head ----
        v_sb = const.tile([P, NT * D * H], F32, tag="v_sb")
        # packed transposed K/Q: partition 32*g + d holds head h = 2*g + hh
        k_t = const.tile([P, 2 * N], F32, tag="k_t")      # [32g+d, hh*N + m]
        q_t = const.tile([P, 2 * LCORE], F32, tag="q_t")  # [32g+d, hh*LCORE + l]

        for t in range(NT):
            kv_nat = setup.tile([P, 2 * D * H], F32, tag="kv_nat")
            nc.sync.dma_start(kv_nat[:], qkv[t * P : (t + 1) * P, D * H : 3 * D * H])
            nc.vector.tensor_copy(
                v_sb[:, t * D * H : (t + 1) * D * H], kv_nat[:, D * H : 2 * D * H]
            )
            # one [128,128] transpose per hh gathers heads {hh, hh+2, hh+4,
            # hh+6}: out partition 32*g + d <- column d*H + (2g + hh)
            kv_v = kv_nat[:, 0 : D * H].rearrange("p (d h) -> p h d", h=H)
            pst = ps_tr.tile([P, 2 * P], F32, tag="pst")
            stage = setup.tile([P, 2 * P], F32, tag="stage")
            for hh in range(2):
                nc.vector.tensor_copy(
                    stage[:, hh * P : (hh + 1) * P], kv_v[:, hh::2, :]
                )
                nc.tensor.transpose(
                    pst[:, hh * P : (hh + 1) * P],
                    stage[:, hh * P : (hh + 1) * P],
                    ident[:],
                )
            nc.scalar.copy(
                k_t[:].rearrange("p (hh m) -> p hh m", hh=2)[:, :, t * P : (t + 1) * P],
                pst[:].rearrange("p (hh c) -> p hh c", hh=2),
            )

        for i in range(NLT):
            q_nat = setup.tile([P, D * H], F32, tag="q_nat")
            nc.sync.dma_start(q_nat[:], q_in[rows(i), :])
            q_v = q_nat[:].rearrange("p (d h) -> p h d", h=H)
            pst = ps_tr.tile([P, 2 * P], F32, tag="pst")
            stage = setup.tile([P, 2 * P], F32, tag="stage")
            for hh in range(2):
                nc.vector.tensor_copy(
                    stage[:, hh * P : (hh + 1) * P], q_v[:, hh::2, :]
                )
                nc.tensor.transpose(
                    pst[:, hh * P : (hh + 1) * P],
                    stage[:, hh * P : (hh + 1) * P],
                    ident[:],
                )
            nc.scalar.copy(
                q_t[:].rearrange("p (hh l) -> p hh l", hh=2)[
                    :, :, i * P : (i + 1) * P
                ],
                pst[:].rearrange("p (hh c) -> p hh c", hh=2),
            )

        # per-core running stats
        sth_all = smalls.tile([P, NLT * H], F32, tag="sth")     # sum tanh
        hr_all = smalls.tile([P, NLT * H], F32, tag="hr_all")   # 0.5/denom
        vraw = smalls.tile([P, NLT * D * H], F32, tag="vraw")   # unscaled V_att

        for i in range(NLT):
            den = smalls.tile([P, NMH * H], F32, tag="den")
            sth = smalls.tile([P, NMH * H], F32, tag="sthp")
            psv = ps_v.tile([P, H * D], F32, tag="psv")
            w_tiles = []
            for s in range(NMH):
                hsb = hpool.tile([P, MH * H], F32, tag="hsb")
                gsb = gpool.tile([P, MH * H], F32, tag="gsb")
                wf = wpool.tile([P, MH * H], F32, tag="wf")   # exp -> W slabs
                uth0 = upool.tile([P, 4 * MH], F32, tag="uth")
                uth1 = upool.tile([P, 4 * MH], F32, tag="uth")
                uths = [uth0, uth1]
                w_tiles.append(wf)
                nc.sync.dma_start(
                    gsb[:], g_in[rows(i), s * MH * H : (s + 1) * MH * H]
                )
                # mm1 (head pairs share a 2-bank psum tile) + wide 2-op clip
                # into the interleaved layout via a 3D output view
                hsb_r = hsb[:].rearrange("p (m h) -> p h m", h=H)
                for g in range(4):
                    mm = ps_mm1.tile([P, 2 * MH], F32, tag="mm1")
                    for hh in range(2):
                        nc.tensor.matmul(
                            mm[:, hh * MH : (hh + 1) * MH],
                            q_t[
                                D * g : D * (g + 1),
                                hh * LCORE + i * P : hh * LCORE + (i + 1) * P,
                            ],
                            k_t[
                                D * g : D * (g + 1),
                                hh * N + s * MH : hh * N + (s + 1) * MH,
                            ],
                            start=True,
                            stop=True,
                            tile_position=(D * g, 0),
                        )
                    nc.vector.tensor_scalar(
                        out=hsb_r[:, 2 * g : 2 * g + 2, :],
                        in0=mm[:],
                        scalar1=CLIP,
                        scalar2=-CLIP,
                        op0=ALU.min,
                        op1=ALU.max,
                    )
                # H_hat = clip + E via accumulate DMA (CCE adder caps at 2048
                # elements per descriptor row -> chunk)
                if "no_accum" not in variant:
                    for o in range(0, MH * H, 2048):
                        nc.gpsimd.dma_start(
                            hsb[:, o : o + 2048],
                            e_in[rows(i), s * MH * H + o : s * MH * H + o + 2048],
                            accum_op=ALU.add,
                        )
                if "no_hout" not in variant:
                    nc.sync.dma_start(
                        h_out[rows(i), s * MH * H : (s + 1) * MH * H], hsb[:]
                    )
                # per-head exp/tanh: strided-in -> flat slab out, fused
                # accum_out row sums (denominator and degrees for free)
                gsb_r = gsb[:].rearrange("p (m h) -> p h m", h=H)
                for h in range(H):
                    nc.scalar.activation(
                        wf[:, h * MH : (h + 1) * MH],
                        hsb[:, h::H],
                        AF.Exp,
                        accum_out=den[:, s * H + h : s * H + h + 1],
                    )
                    if h % 2 == 0:
                        # tanh for a head pair in one op: 3D strided in,
                        # contiguous pair-slab out (sums recovered on DVE)
                        nc.scalar.activation(
                            uths[h // 4][:, (h % 4) * MH : (h % 4 + 2) * MH],
                            gsb_r[:, h : h + 2, :],
                            AF.Tanh,
                            scale=0.5,
                        )
                for q in range(2):
                    u_r = uths[q][:].rearrange("p (h m) -> p h m", h=4)
                    nc.vector.reduce_sum(
                        sth[:, s * H + 4 * q : s * H + 4 * (q + 1)],
                        u_r,
                        axis=mybir.AxisListType.X,
                    )
                # W = exp*(1+tanh) = exp + exp*tanh, built on gpsimd in
                # half-tile chunks (W lands in-place in wf; uth is scratch)
                if "no_gpsw" not in variant:
                    for q in range(2):
                        cs = slice(q * 4 * MH, (q + 1) * 4 * MH)
                        u = uths[q]
                        nc.gpsimd.tensor_tensor(u[:], u[:], wf[:, cs], ALU.mult)
                        nc.gpsimd.tensor_tensor(wf[:, cs], wf[:, cs], u[:], ALU.add)
                # transposes of flat W chunks + mm2 (no denom dependency)
                for h in (() if "no_pe2" in variant else range(H)):
                    pst = ps_tr.tile([P, NCH * P], F32, tag="pst")
                    for c in range(NCH):
                        nc.tensor.transpose(
                            pst[:, c * P : (c + 1) * P],
                            wf[:, h * MH + c * P : h * MH + (c + 1) * P],
                            ident[:],
                        )
                    at_sb = atpool.tile([P, NCH * P], F32, tag="at_sb")
                    nc.any.tensor_copy(at_sb[:], pst[:])
                    for c in range(NCH):
                        t = s * NCH + c  # global m tile index
                        # one accumulation group spans the whole psv bank
                        nc.tensor.matmul(
                            psv[:, h * D : (h + 1) * D],
                            at_sb[:, c * P : (c + 1) * P],
                            v_sb[:, t * D * H + h : (t + 1) * D * H : H],
                            start=(s == 0 and h == 0 and c == 0),
                            stop=(s == NMH - 1 and h == H - 1 and c == NCH - 1),
                        )

            # 0.5/denom over both halves
            denom = smalls.tile([P, H], F32, tag="denom")
            nc.vector.tensor_add(denom[:], den[:, 0:H], den[:, H : 2 * H])
            nc.vector.tensor_add(
                sth_all[:, i * H : (i + 1) * H], sth[:, 0:H], sth[:, H : 2 * H]
            )
            half_r = hr_all[:, i * H : (i + 1) * H]
            nc.vector.reciprocal(half_r, denom[:])
            nc.vector.tensor_scalar_mul(half_r, half_r, 0.5)

            # A_tild = W * (0.5/denom), interleaved for DRAM ([128,2048] chunks)
            QW = MH * H // 2
            for s in (() if "no_aout" in variant else range(NMH)):
                wf = w_tiles[s]
                for q in range(2):
                    asb = apool.tile([P, QW], F32, tag="asb")
                    mq = MH // 2  # m-columns per chunk
                    for h in range(H):
                        nc.any.tensor_scalar(
                            out=asb[:, h::H],
                            in0=wf[:, h * MH + q * mq : h * MH + (q + 1) * mq],
                            scalar1=half_r[:, h : h + 1],
                            scalar2=None,
                            op0=ALU.mult,
                        )
                    nc.sync.dma_start(
                        a_out[rows(i), s * MH * H + q * QW : s * MH * H + (q + 1) * QW],
                        asb[:],
                    )
            # evict raw V_att (scaled at the end)
            if "no_pe2" in variant:
                nc.vector.tensor_copy(
                    vraw[:, i * D * H : (i + 1) * D * H], w_tiles[0][:, 0 : D * H]
                )
            else:
                nc.scalar.copy(vraw[:, i * D * H : (i + 1) * D * H], psv[:])

        # ---- final: V_att = vraw * (0.5/denom) * log1p(N/2 + 0.5*sum tanh) ----
        lnsc = smalls.tile([P, NLT * H], F32, tag="lnsc")
        nc.scalar.activation(lnsc[:], sth_all[:], AF.Ln, scale=0.5, bias=ln_bias[:])
        nc.vector.tensor_mul(lnsc[:], lnsc[:], hr_all[:])
        vout_sb = smalls.tile([P, NLT * D * H], F32, tag="vout_sb")
        for i in range(NLT):
            for h in range(H):
                nc.vector.tensor_scalar(
                    out=vout_sb[:, i * D * H + h : (i + 1) * D * H : H],
                    in0=vraw[:, i * D * H + h * D : i * D * H + (h + 1) * D],
                    scalar1=lnsc[:, i * H + h : i * H + h + 1],
                    scalar2=None,
                    op0=ALU.mult,
                )
            nc.sync.dma_start(
                v_out[rows(i), :], vout_sb[:, i * D * H : (i + 1) * D * H]
            )


_programs = {}


def build_program(repeat=1, alias_rows=False, variant=()):
    key = (repeat, alias_rows, tuple(sorted(variant)))
    if key in _programs:
        return _programs[key]
    RR = P if alias_rows else LCORE
    nc = bacc.Bacc("TRN2", target_bir_lowering=False, debug=False, num_devices=8)
    qkv = nc.dram_tensor("qkv", [N, 3 * D * H], F32, kind="ExternalInput").ap()
    q_in = nc.dram_tensor("q_in", [RR, D * H], F32, kind="ExternalInput").ap()
    e_in = nc.dram_tensor("e_in", [RR, N * H], F32, kind="ExternalInput").ap()
    g_in = nc.dram_tensor("g_in", [RR, N * H], F32, kind="ExternalInput").ap()
    h_out = nc.dram_tensor("h_out", [RR, N * H], F32, kind="ExternalOutput").ap()
    a_out = nc.dram_tensor("a_out", [RR, N * H], F32, kind="ExternalOutput").ap()
    v_out = nc.dram_tensor("v_out", [RR, D * H], F32, kind="ExternalOutput").ap()
    with tile.TileContext(nc) as tc:
        egt_body(tc, (h_out, a_out, v_out), (qkv, q_in, e_in, g_in), repeat=repeat,
                 alias_rows=alias_rows, variant=variant)
    nc.compile()
    _programs[key] = nc
    return nc


def kernel(QKV, E, G, repeat=1, _timing_out=None):
    QKV = np.ascontiguousarray(np.asarray(QKV, dtype=np.float32))
    E = np.ascontiguousarray(np.asarray(E, dtype=np.float32))
    G = np.ascontiguousarray(np.asarray(G, dtype=np.float32))
    assert QKV.shape == (B, N, 3 * D * H)
    assert E.shape == (B, N, N, H) and G.shape == (B, N, N, H)

    nc = build_program(repeat)
    in_maps = []
    for c in range(8):
        b, lh = c // 2, c % 2
        sl = slice(lh * LCORE, (lh + 1) * LCORE)
        in_maps.append(
            {
                "qkv": QKV[b],
                "q_in": np.ascontiguousarray(QKV[b, sl, 0 : D * H]),
                "e_in": np.ascontiguousarray(E[b, sl].reshape(LCORE, N * H)),
                "g_in": np.ascontiguousarray(G[b, sl].reshape(LCORE, N * H)),
            }
        )
    import time

    t0 = time.perf_counter()
    res = run_bass_kernel_spmd(nc, in_maps, list(range(8))).results
    t1 = time.perf_counter()
    if _timing_out is not None:
        _timing_out.append(t1 - t0)

    V_att = np.empty((B, N, D * H), np.float32)
    H_hat = np.empty((B, N, N, H), np.float32)
    A_tild = np.empty((B, N, N, H), np.float32)
    for c, r in enumerate(res):
        b, lh = c // 2, c % 2
        sl = slice(lh * LCORE, (lh + 1) * LCORE)
        V_att[b, sl] = r["v_out"]
        H_hat[b, sl] = r["h_out"].reshape(LCORE, N, H)
        A_tild[b, sl] = r["a_out"].reshape(LCORE, N, H)
    return V_att, H_hat, A_tild
